# revision 4
# baseline (speedup 1.0000x reference)
"""Trainium2 Bass kernel for DecoderOnlyAspire segment-reduce problem.

Computes, for hidden [B=8, S=4096, D=1536] f32:
  - doc_reps  [B, D]    : last-token pooling (per reference semantics)
  - sent_reps [B, M, D] : per-sentence segment means (M = 24)

Strategy: data-parallel over batch across 8 NeuronCores (one example per
core; no cross-core communication).  On each core both outputs come from
PE matmuls: out[M+1, D] = W.T @ hidden_b where W [S, M+1] is the exact
{0,1} one-hot of sent_ids (col M = last-token indicator); the division
by segment count is folded into a per-partition scalar multiply on the
way out of PSUM.

To hit the memory roofline, the host losslessly recodes hidden into a
bf16 hi/lo pair (hi = bf16(h), lo = bf16(h - hi); |h - hi - lo| <=
~2^-18 |h|) laid out so every DMA reads large contiguous per-partition
chunks, and the PE runs bf16 matmuls (1 cycle/row instead of fp32's 4).
Token t lives at partition p = t // 32, slot q = t % 32; W rows are
permuted identically, so the matmul contraction stays consistent.
"""

import os

import numpy as np

B, S, D, M = 8, 4096, 1536, 24
P = 128              # SBUF partitions
Q = S // P           # 32 token slots per partition
MM = M + 1           # segment columns + doc (last-token) column
NBANK = 512          # fp32 elements per PSUM bank
NJ = D // NBANK      # 3 bank-column chunks
QC = 8               # token slots per DMA chunk
NCH = Q // QC        # 4 chunks

_PROGRAM = None
_LAST_RESULTS = None  # BassKernelResults of the most recent run (for test harness)


def _build_program():
    import concourse.bacc as bacc
    import concourse.tile as tile
    from concourse import mybir

    nc = bacc.Bacc("TRN2", target_bir_lowering=False, debug=False)

    # [p, q, a, d] bf16: a=0 hi, a=1 lo; token t = p*Q + q
    h_in = nc.declare_dram_parameter(
        "hidden_b", [P, Q * 2 * D], mybir.dt.bfloat16, isOutput=False
    )
    w_in = nc.declare_dram_parameter("w_b", [P, Q * MM], mybir.dt.bfloat16, isOutput=False)
    inv_in = nc.declare_dram_parameter("inv_b", [MM, 1], mybir.dt.float32, isOutput=False)
    out_ext = nc.declare_dram_parameter("out_b", [MM, D], mybir.dt.float32, isOutput=True)

    h_view = h_in[:].rearrange("p (q a d) -> p q a d", q=Q, a=2)

    with tile.TileContext(nc) as tc:
        with (
            tc.tile_pool(name="w", bufs=1) as wpool,
            tc.tile_pool(name="h", bufs=3) as hpool,
            tc.tile_pool(name="ps", bufs=1, space="PSUM") as pspool,
            tc.tile_pool(name="o", bufs=1) as opool,
        ):
            w_tile = wpool.tile([P, Q, MM], mybir.dt.bfloat16)
            nc.sync.dma_start(w_tile[:], w_in[:].rearrange("p (q m) -> p q m", q=Q))
            inv_tile = wpool.tile([MM, 1], mybir.dt.float32, tag="inv")
            nc.sync.dma_start(inv_tile[:], inv_in[:])

            psum_t = pspool.tile([MM, D], mybir.dt.float32)
            for c in range(NCH):
                h_t = hpool.tile([P, QC, 2, D], mybir.dt.bfloat16)
                nc.sync.dma_start(h_t[:], h_view[:, c * QC : (c + 1) * QC, :, :])
                for q in range(QC):
                    tg = c * QC + q
                    for a in range(2):
                        for j in range(NJ):
                            nc.tensor.matmul(
                                psum_t[:, j * NBANK : (j + 1) * NBANK],
                                w_tile[:, tg, :],
                                h_t[:, q, a, j * NBANK : (j + 1) * NBANK],
                                start=(tg == 0 and a == 0),
                                stop=(tg == Q - 1 and a == 1),
                            )

            out_t = opool.tile([MM, D], mybir.dt.float32)
            nc.vector.tensor_scalar_mul(out_t[:], psum_t[:], inv_tile[:, 0:1])
            nc.sync.dma_start(out_ext[:], out_t[:])

    nc.compile()
    return nc


def _get_program():
    global _PROGRAM
    if _PROGRAM is None:
        _PROGRAM = _build_program()
    return _PROGRAM


def _prepare_inputs(hidden, attn_mask, sent_ids):
    """Host-side lossless recode: bf16 hi/lo split + layout permute + W."""
    import ml_dtypes

    bf16 = ml_dtypes.bfloat16

    # Last-token index per example (same semantics as the reference).
    left_padding = int(attn_mask[:, -1].sum()) == B
    seq_lengths = attn_mask.sum(axis=1) - 1  # [B]
    if left_padding:
        idx = np.full(B, S - 1, dtype=np.int64)
    else:
        idx = seq_lengths.astype(np.int64)

    hi = hidden.astype(bf16)
    lo = (hidden - hi.astype(np.float32)).astype(bf16)
    # [B, S, D] -> [B, P, Q, 2, D] with token t = p*Q + q
    pair = np.stack([hi, lo], axis=2)  # [B, S, 2, D]
    h_dev = np.ascontiguousarray(
        pair.reshape(B, P, Q, 2, D).reshape(B, P, Q * 2 * D)
    )

    w = np.zeros((B, S, MM), dtype=bf16)
    tok = np.arange(S)
    inv = np.zeros((B, MM, 1), dtype=np.float32)
    for b in range(B):
        w[b, tok, sent_ids[b]] = 1
        w[b, idx[b], M] = 1
        counts = np.bincount(sent_ids[b], minlength=M)
        inv[b, :M, 0] = (1.0 / np.maximum(counts, 1)).astype(np.float32)
        inv[b, M, 0] = 1.0
    w_dev = np.ascontiguousarray(w.reshape(B, P, Q * MM))
    return h_dev, w_dev, inv


def kernel(hidden, attn_mask, sent_ids, max_sents):
    global _LAST_RESULTS
    from concourse.bass_utils import run_bass_kernel_spmd

    hidden = np.ascontiguousarray(np.asarray(hidden, dtype=np.float32))
    attn_mask = np.asarray(attn_mask).astype(np.int32)
    sent_ids = np.asarray(sent_ids).astype(np.int32)
    m = int(max_sents)
    assert hidden.shape == (B, S, D) and m == M

    h_dev, w_dev, inv = _prepare_inputs(hidden, attn_mask, sent_ids)

    nc = _get_program()
    in_maps = [
        {"hidden_b": h_dev[b], "w_b": w_dev[b], "inv_b": inv[b]} for b in range(B)
    ]
    trace = bool(os.environ.get("KERNEL_TRACE"))
    kwargs = {}
    if trace:
        base = os.environ.get("KERNEL_TRACE_DIR")
        if base:
            import tempfile

            os.makedirs(base, exist_ok=True)
            kwargs["tmpdir"] = tempfile.mkdtemp(dir=base)
        if os.environ.get("KERNEL_TRACE_CORES"):
            kwargs["trace_cores"] = [
                int(c) for c in os.environ["KERNEL_TRACE_CORES"].split(",")
            ]
    res = run_bass_kernel_spmd(nc, in_maps, list(range(B)), trace=trace, **kwargs)
    _LAST_RESULTS = res
    out = np.stack([res.results[b]["out_b"] for b in range(B)])  # [B, MM, D]
    doc_reps = out[:, M, :].copy()
    sent_reps = out[:, :M, :].copy()
    return doc_reps, sent_reps
